# revision 10
# baseline (speedup 1.0000x reference)
"""CrossAttentionNoGate Trainium2 kernel.

Shards the MSA-row dim S (=64) across 8 NeuronCores (8 rows/core, fully
data-parallel, no collectives).  The whole per-core computation is emitted as
ONE flat software pipeline over score tiles u (16 per s-row: 2 head-groups x
4 kv-blocks x 2 head-pair halves), with att/den consumption lagging scores/exp
by one tile so no engine ever waits in-order on a cross-engine round trip:

  u:   [PE] scores tile u (+bias identity-MMs for groups < SPLIT)
       [ACT] p = exp(scores + maskterm)     (mask via per-partition bias)
       [DVE] p *= exp(bias)   (groups >= SPLIT; f16 table precomputed on host)
       [PE] att/den col-tiled matmuls for tile u-1
  per-row hooks (at the right lag points): softmax normalize per group,
  next-row q/k/v projections (split into 1-bank pieces), output projection,
  +bo, DMA out.

PSUM: 2x [128,1024] score ring (4 banks) + 4x [128,512] misc ring (4 banks)
whose strict allocation order (att0,den0,q,q2,att1,den1,k,k2,v,v2,po,po2 per
row) makes every ring-reuse wait benign.

All matmuls run as float32r (1 cycle/row for N>=256).
Self-contained: hardcodes all shapes; host side only reshapes/shards.
"""

import os
import sys

import numpy as np

if "/opt/trn_rl_repo" not in sys.path:
    sys.path.insert(0, "/opt/trn_rl_repo")

import concourse.bass as bass
import concourse.bacc as bacc
import concourse.tile as tile
from concourse import mybir
from concourse.bass_utils import run_bass_kernel_spmd

S, Q, KV, C, H, D = 64, 512, 512, 256, 8, 32
NCORES = 8
SLOC = S // NCORES          # 8 s-rows per core
HD = H * D                  # 256
SPLIT = int(os.environ.get("K_SPLIT", "1"))    # groups < SPLIT: PE bias
ABL = os.environ.get("K_ABL", "")       # timing-only ablations (break numerics)
OFF_WQ, OFF_WK, OFF_WV, OFF_WO = 0, 512, 1024, 1536
OFF_BO, OFF_MV, OFF_ID = 2048, 3072, 3104
OFF_BIAS = OFF_ID + 128
BLOB_COLS = OFF_BIAS + SPLIT * (H // 2) * 4 * Q
EB_COLS = (2 - SPLIT) * (H // 2) * 4 * Q
MD_COLS = SLOC * 4 * 32          # mask col f16, 32x-replicated per (s,b)
F32 = mybir.dt.float32
F32R = mybir.dt.float32r
F16 = mybir.dt.float16
EXP = mybir.ActivationFunctionType.Exp

LAST_RESULT = None          # test.py reads exec_time/profile from here
_COMPILED = None


def build_nc(repeat=1):
    from contextlib import ExitStack

    nc = bacc.Bacc("TRN2", target_bir_lowering=False, debug=False,
                   enable_asserts=False, num_devices=NCORES)
    blob = nc.declare_dram_parameter("blob", [128, BLOB_COLS], F32R, isOutput=False)
    ebD = nc.declare_dram_parameter(
        "eb", [128, EB_COLS + MD_COLS], F16, isOutput=False)
    xqT = nc.declare_dram_parameter("xqT", [SLOC, C, Q], F32R, isOutput=False)
    xkvT = nc.declare_dram_parameter("xkvT", [SLOC, C, KV], F32R, isOutput=False)
    out = nc.declare_dram_parameter("out", [SLOC, Q, C], F32, isOutput=True)

    with tile.TileContext(nc) as tc, ExitStack() as ctx:
        singles = ctx.enter_context(tc.tile_pool(name="singles", bufs=1))
        qT_pool = ctx.enter_context(tc.tile_pool(name="qTp", bufs=2))
        kT_pool = ctx.enter_context(tc.tile_pool(name="kTp", bufs=2))
        v_pool = ctx.enter_context(tc.tile_pool(name="vp", bufs=2))
        p_pool = ctx.enter_context(tc.tile_pool(name="pp", bufs=6))
        recip_pool = ctx.enter_context(tc.tile_pool(name="rp", bufs=2))
        oT_pool = ctx.enter_context(tc.tile_pool(name="oTp", bufs=2))
        out_pool = ctx.enter_context(tc.tile_pool(name="outp", bufs=2))

        blob_t = singles.tile([128, BLOB_COLS], F32R)
        nc.sync.dma_start(out=blob_t[:], in_=blob[:])
        wq_t = blob_t[:, OFF_WQ:OFF_WQ + 512].rearrange("p (c m) -> p c m", c=2)
        wk_t = blob_t[:, OFF_WK:OFF_WK + 512].rearrange("p (c m) -> p c m", c=2)
        wv_t = blob_t[:, OFF_WV:OFF_WV + 512].rearrange("p (c m) -> p c m", c=2)
        wo_t = blob_t[:, OFF_WO:OFF_WO + 512].rearrange("p (c m) -> p c m", c=2)
        bo4_t = blob_t[:, OFF_BO:OFF_BO + 1024].bitcast(F32)
        maskv_t = blob_t[:, OFF_MV:OFF_MV + SLOC * 4].bitcast(F32)
        ident_t = blob_t[:, OFF_ID:OFF_ID + 128]
        if SPLIT:
            biasC = blob_t[:, OFF_BIAS:].rearrange(
                "p (h b q) -> p h b q", h=4 * SPLIT, b=4)
        if EB_COLS:
            eb_s = singles.tile([128, EB_COLS], F16)
            nc.sync.dma_start(out=eb_s[:], in_=ebD[:, :EB_COLS])
            eb_t = eb_s[:].rearrange(
                "p (h b q) -> p h b q", h=4 * (2 - SPLIT), b=4)
        md_s = singles.tile([128, SLOC, 4, 32], F16)
        nc.sync.dma_start(
            out=md_s[:], in_=ebD[:, EB_COLS:].rearrange(
                "p (s b m) -> p s b m", s=SLOC, b=4))
        zeros_t = singles.tile([128, 128], F16)
        nc.vector.memset(zeros_t[:], 0.0)
        xq_all = singles.tile([128, SLOC, 2, Q], F32R)
        nc.sync.dma_start(
            out=xq_all[:], in_=xqT[:].rearrange("s (c p) q -> p s c q", p=128))
        xkv_all = singles.tile([128, SLOC, 2, KV], F32R)
        nc.sync.dma_start(
            out=xkv_all[:], in_=xkvT[:].rearrange("s (c p) q -> p s c q", p=128))

        ps_sc = ctx.enter_context(
            tc.tile_pool(name="ps_sc", bufs=2, space="PSUM"))
        ps_ms = ctx.enter_context(
            tc.tile_pool(name="ps_ms", bufs=4, space="PSUM"))

        # ---------- projection pieces (each uses one 1-bank misc tile) ----
        def proj_qk_piece(s, hc, w_t, x_t, dstT):
            ps = ps_ms.tile([128, 512], F32, tag="ms", name=f"pj{hc}")
            for cc in range(2):
                nc.tensor.matmul(
                    ps[:], w_t[:, cc, 128 * hc:128 * hc + 128],
                    x_t[:, cc, :], start=(cc == 0), stop=(cc == 1))
            nc.vector.tensor_copy(dstT[:, hc, :], ps[:])

        def proj_v_piece(s, pr, v_t):
            xkv_t = xkv_all[:, s]
            ps = ps_ms.tile([128, 512], F32, tag="ms", name=f"pv{pr}")
            for bb in range(2):
                b2 = 2 * pr + bb
                for cc in range(2):
                    nc.tensor.matmul(
                        ps[:, 256 * bb:256 * bb + 256],
                        xkv_t[:, cc, 128 * b2:128 * b2 + 128],
                        wv_t[:, cc, :], start=(cc == 0), stop=(cc == 1))
            for bb in range(2):
                b2 = 2 * pr + bb
                nc.vector.tensor_scalar_mul(
                    v_t[:, b2, :], ps[:, 256 * bb:256 * bb + 256],
                    maskv_t[:, s * 4 + b2:s * 4 + b2 + 1])

        def emit_proj(s):
            qT_t = qT_pool.tile([128, 2, Q], F32R, tag="qT")
            kT_t = kT_pool.tile([128, 2, KV], F32R, tag="kT")
            v_t = v_pool.tile([128, 4, HD], F16, tag="v")
            return qT_t, kT_t, v_t

        # ---------- pipeline state ----------
        s_list = [s for _ in range(repeat) for s in range(SLOC)]
        nrow = len(s_list)

        # prologue: projections for row 0
        proj_tiles = {0: emit_proj(s_list[0])}
        for hc in range(2):
            proj_qk_piece(s_list[0], hc, wq_t, xq_all[:, s_list[0]], proj_tiles[0][0])
        for hc in range(2):
            proj_qk_piece(s_list[0], hc, wk_t, xkv_all[:, s_list[0]], proj_tiles[0][1])
        for pr in range(2):
            proj_v_piece(s_list[0], pr, proj_tiles[0][2])

        row_state = {}          # per-row: att/den/oT tiles

        def emit_scores(si, s, g, b, half, qT_t, kT_t):
            sc = ps_sc.tile([128, 1024], F32, tag="sc", name="sc")
            for jj in range(2):
                j = 2 * half + jj
                h = 4 * g + j
                if g < SPLIT and ABL != "bias":
                    nc.tensor.matmul(
                        sc[:, 512 * jj:512 * jj + 512],
                        ident_t[:], biasC[:, h, b, :],
                        start=True, stop=False, skip_group_check=True)
            for jj in range(2):
                j = 2 * half + jj
                nc.tensor.matmul(
                    sc[:, 512 * jj:512 * jj + 512],
                    kT_t[32 * j:32 * j + 32, g, 128 * b:128 * b + 128],
                    qT_t[32 * j:32 * j + 32, g, :],
                    start=(g >= SPLIT) or ABL == "bias", stop=True,
                    skip_group_check=True, tile_position=(32 * j, 0))
            p = p_pool.tile([128, 1024], F16, tag="p", name="p")
            if ABL == "exp":
                nc.vector.tensor_copy(
                    p[:, :8], sc[:, :8])  # tiny drain keeps sc ring moving
            else:
                nc.scalar.activation(out=p[:], in_=sc[:], func=EXP)
            if g >= SPLIT and ABL != "mul":
                hh = 4 * (g - SPLIT) + 2 * half
                pm = p_pool.tile([128, 1024], F16, tag="p", name="pm")
                nc.vector.tensor_mul(
                    pm[:].rearrange("p (a q) -> p a q", a=2),
                    p[:].rearrange("p (a q) -> p a q", a=2),
                    eb_t[:, hh:hh + 2, b, :])
                p = pm
            return p

        def emit_attden(si, ps_, g, b, half, p, v_t):
            pb = b
            st = row_state[si]
            if ABL == "attden":
                if b == 0 and half == 0:
                    st[g] = (
                        ps_ms.tile([128, 512], F32, tag="ms", name="att"),
                        ps_ms.tile([128, 512], F32, tag="ms", name="den"))
                    att_t, den_t = st[g]
                    nc.tensor.matmul(att_t[:], zeros_t[:], p[:, :512],
                                     start=True, stop=(g == 9),
                                     skip_group_check=True)
                    nc.tensor.matmul(den_t[:], zeros_t[:], p[:, :512],
                                     start=True, stop=(g == 9),
                                     skip_group_check=True)
                return
            if b == 0 and half == 0:
                st[g] = (
                    ps_ms.tile([128, 512], F32, tag="ms", name="att"),
                    ps_ms.tile([128, 512], F32, tag="ms", name="den"))
                att_t, den_t = st[g]
                nc.tensor.matmul(att_t[:], zeros_t[:], p[:, :512],
                                 start=True, stop=False, skip_group_check=True)
                nc.tensor.matmul(den_t[:], zeros_t[:], p[:, :512],
                                 start=True, stop=False, skip_group_check=True)
            att_t, den_t = st[g]
            last = (b == 3 and half == 1)
            for jj in range(2):
                j = 2 * half + jj
                rhs = p[:, 512 * jj:512 * jj + 512]
                nc.tensor.matmul(
                    att_t[32 * j:32 * j + 32, :],
                    v_t[:, b, 32 * (4 * g + j):32 * (4 * g + j) + 32], rhs,
                    start=False, stop=(last and jj == 1),
                    skip_group_check=True, tile_position=(0, 32 * j))
            for jj in range(2):
                j = 2 * half + jj
                rhs = p[:, 512 * jj:512 * jj + 512]
                nc.tensor.matmul(
                    den_t[32 * j:32 * j + 32, :], md_s[:, ps_, pb, :], rhs,
                    start=False, stop=(last and jj == 1),
                    skip_group_check=True, tile_position=(0, 32 * j))

        def emit_norm(si, g):
            st = row_state[si]
            att_t, den_t = st[g]
            if g == 0:
                st["oT"] = oT_pool.tile([128, 1024], F32R, tag="oT", name="oT")
            oT_t = st["oT"]
            recip_t = recip_pool.tile([128, 512], F32, tag="recip")
            nc.vector.reciprocal_approx_fast(out=recip_t[:], in_=den_t[:])
            nc.vector.tensor_mul(oT_t[:, 512 * g:512 * g + 512],
                                 att_t[:], recip_t[:])

        def emit_outproj(si, s):
            st = row_state[si]
            oT_t = st["oT"]
            out_t = out_pool.tile([128, 4 * C], F32, tag="out")
            for pq in range(2):
                ps = ps_ms.tile([128, 512], F32, tag="ms", name=f"po{pq}")
                for qq in range(2):
                    qb = 2 * pq + qq
                    for c in range(2):
                        nc.tensor.matmul(
                            ps[:, 256 * qq:256 * qq + 256],
                            oT_t[:, 512 * c + 128 * qb:512 * c + 128 * qb + 128],
                            wo_t[:, c, :], start=(c == 0), stop=(c == 1))
                nc.vector.tensor_add(
                    out_t[:, 512 * pq:512 * pq + 512], ps[:],
                    bo4_t[:, 512 * pq:512 * pq + 512])
            nc.gpsimd.dma_start(
                out=out[s].rearrange("(b p) c -> p b c", p=128),
                in_=out_t[:].rearrange("p (b c) -> p b c", b=4))

        pend = None             # (si, s, g, b, half, p, v_t)

        def process_pend():
            nonlocal pend
            if pend is None:
                return
            psi, ps_, pg, pb, ph, pp, pv = pend
            emit_attden(psi, ps_, pg, pb, ph, pp, pv)
            pend = None
            u = ((pg * 4) + pb) * 2 + ph
            if u == 1 and psi > 0 and (psi - 1) in row_state:
                emit_outproj(psi - 1, s_list[psi - 1])
                del row_state[psi - 1]
            elif u == 7:
                emit_norm(psi, 0)
                if psi + 1 < nrow:
                    sn = s_list[psi + 1]
                    for hc in range(2):
                        proj_qk_piece(sn, hc, wq_t, xq_all[:, sn],
                                      proj_tiles[psi + 1][0])
            elif u == 11:
                if psi + 1 < nrow:
                    sn = s_list[psi + 1]
                    for hc in range(2):
                        proj_qk_piece(sn, hc, wk_t, xkv_all[:, sn],
                                      proj_tiles[psi + 1][1])
            elif u == 15:
                emit_norm(psi, 1)
                if psi + 1 < nrow:
                    sn = s_list[psi + 1]
                    for pr in range(2):
                        proj_v_piece(sn, pr, proj_tiles[psi + 1][2])

        for si, s in enumerate(s_list):
            row_state[si] = {}
            qT_t, kT_t, v_t = proj_tiles[si]
            if si + 1 < nrow:
                proj_tiles[si + 1] = emit_proj(s_list[si + 1])
            for g in range(2):
                for b in range(4):
                    for half in range(2):
                        p = emit_scores(si, s, g, b, half, qT_t, kT_t)
                        process_pend()
                        pend = (si, s, g, b, half, p, v_t)
            if si - 1 >= 0:
                del proj_tiles[si - 1]
        process_pend()
        emit_outproj(nrow - 1, s_list[nrow - 1])
        del row_state[nrow - 1]

    nc.compile()
    return nc


def _get_compiled():
    global _COMPILED
    if _COMPILED is None:
        _COMPILED = build_nc()
    return _COMPILED


def prep_in_maps(input_q, input_kv, mask, bias, Wq, Wkv, Wo, bo):
    input_q = np.asarray(input_q, dtype=np.float32)
    input_kv = np.asarray(input_kv, dtype=np.float32)
    mask = np.asarray(mask, dtype=np.float32)
    bias = np.asarray(bias, dtype=np.float32)
    Wq = np.asarray(Wq, dtype=np.float32)
    Wkv = np.asarray(Wkv, dtype=np.float32)
    Wo = np.asarray(Wo, dtype=np.float32)
    bo = np.asarray(bo, dtype=np.float32)

    # [h, kv, q] bias, packed as [p, h, b, q]; f32 for PE groups, exp-f16 rest
    biasT = np.transpose(bias[0, 0], (0, 2, 1))
    bias_pk = np.ascontiguousarray(
        biasT.reshape(H, 4, 128, Q).transpose(2, 0, 1, 3).reshape(128, H * 4 * Q))
    eb_pk = np.exp(bias_pk[:, 4 * SPLIT * 4 * Q:]).astype(np.float16)
    if EB_COLS == 0:
        eb_pk = np.zeros((128, 0), np.float16)

    def chunks2(w):  # [C, M] -> [p, (c m)] with 128-row C-chunks
        return w.reshape(2, 128, w.shape[1]).transpose(1, 0, 2).reshape(128, -1)

    wq_s = chunks2(Wq / np.sqrt(np.float32(D)))
    wk_pk = chunks2(Wkv[:, :HD])
    wv_pk = chunks2(Wkv[:, HD:])
    wo_pk = chunks2(Wo)
    bo4 = np.tile(bo[None, :], (128, 4))
    ident = np.eye(128, dtype=np.float32)

    in_maps = []
    for cid in range(NCORES):
        sl = slice(cid * SLOC, (cid + 1) * SLOC)
        xqT = np.ascontiguousarray(np.transpose(input_q[0, sl], (0, 2, 1)))
        xkvT = np.ascontiguousarray(np.transpose(input_kv[0, sl], (0, 2, 1)))
        m = mask[0, sl, 0, 0, :]                       # [SLOC, KV]
        maskcol = m.reshape(SLOC, 4, 128).transpose(2, 0, 1).reshape(128, SLOC * 4)
        md = np.ascontiguousarray(np.broadcast_to(
            maskcol.astype(np.float16)[:, :, None], (128, SLOC * 4, 32))
        ).reshape(128, MD_COLS)
        blob = np.zeros((128, BLOB_COLS), np.float32)
        blob[:, OFF_WQ:OFF_WQ + 512] = wq_s
        blob[:, OFF_WK:OFF_WK + 512] = wk_pk
        blob[:, OFF_WV:OFF_WV + 512] = wv_pk
        blob[:, OFF_WO:OFF_WO + 512] = wo_pk
        blob[:, OFF_BO:OFF_BO + 1024] = bo4
        blob[:, OFF_MV:OFF_MV + SLOC * 4] = maskcol
        blob[:, OFF_ID:OFF_ID + 128] = ident
        if SPLIT:
            blob[:, OFF_BIAS:] = bias_pk[:, :SPLIT * 4 * 4 * Q]
        in_maps.append(dict(
            blob=blob, eb=np.concatenate([eb_pk, md], axis=1),
            xqT=xqT, xkvT=xkvT))

    return in_maps


def kernel(input_q, input_kv, mask, bias, Wq, Wkv, Wo, bo):
    global LAST_RESULT
    nc = _get_compiled()
    in_maps = prep_in_maps(input_q, input_kv, mask, bias, Wq, Wkv, Wo, bo)
    trace = bool(int(os.environ.get("KERNEL_TRACE", "0")))
    LAST_RESULT = run_bass_kernel_spmd(
        nc, in_maps, list(range(NCORES)), trace=trace)
    outs = [LAST_RESULT.results[cid]["out"] for cid in range(NCORES)]
    full = np.concatenate(outs, axis=0)[None]          # [1, S, Q, C]
    return np.ascontiguousarray(full.astype(np.float32))


if __name__ == "__main__":
    rng = np.random.default_rng(0)
    demo = dict(
        input_q=rng.standard_normal((1, S, Q, C), dtype=np.float32),
        input_kv=rng.standard_normal((1, S, KV, C), dtype=np.float32),
        mask=np.ones((1, S, 1, 1, KV), np.float32),
        bias=rng.standard_normal((1, 1, H, Q, KV), dtype=np.float32) * 0.1,
        Wq=rng.standard_normal((C, HD), dtype=np.float32) * 0.06,
        Wkv=rng.standard_normal((C, 2 * HD), dtype=np.float32) * 0.05,
        Wo=rng.standard_normal((HD, C), dtype=np.float32) * 0.02,
        bo=np.zeros((C,), np.float32),
    )
    o = kernel(**demo)
    print("out", o.shape, o.dtype, float(np.abs(o).max()))


# revision 11
# speedup vs baseline: 1.2056x; 1.2056x over previous
"""CrossAttentionNoGate Trainium2 kernel.

Shards the MSA-row dim S (=64) across 8 NeuronCores (8 rows/core, fully
data-parallel, no collectives).  The whole per-core computation is emitted as
ONE flat software pipeline over score tiles u (16 per s-row: 2 head-groups x
4 kv-blocks x 2 head-pair halves), with att/den consumption lagging scores/exp
by one tile so no engine ever waits in-order on a cross-engine round trip:

  u:   [PE] scores tile u (+bias identity-MMs for groups < SPLIT)
       [ACT] p = exp(scores + maskterm)     (mask via per-partition bias)
       [DVE] p *= exp(bias)   (groups >= SPLIT; f16 table precomputed on host)
       [PE] att/den col-tiled matmuls for tile u-1
  per-row hooks (at the right lag points): softmax normalize per group,
  next-row q/k/v projections (split into 1-bank pieces), output projection,
  +bo, DMA out.

PSUM: 2x [128,1024] score ring (4 banks) + 4x [128,512] misc ring (4 banks)
whose strict allocation order (att0,den0,q,q2,att1,den1,k,k2,v,v2,po,po2 per
row) makes every ring-reuse wait benign.

All matmuls run as float32r (1 cycle/row for N>=256).
Self-contained: hardcodes all shapes; host side only reshapes/shards.
"""

import os
import sys

import numpy as np

if "/opt/trn_rl_repo" not in sys.path:
    sys.path.insert(0, "/opt/trn_rl_repo")

import concourse.bass as bass
import concourse.bacc as bacc
import concourse.tile as tile
from concourse import mybir
from concourse.bass_utils import run_bass_kernel_spmd

S, Q, KV, C, H, D = 64, 512, 512, 256, 8, 32
NCORES = 8
SLOC = S // NCORES          # 8 s-rows per core
HD = H * D                  # 256
SPLIT = int(os.environ.get("K_SPLIT", "1"))    # groups < SPLIT: PE bias
ABL = os.environ.get("K_ABL", "")       # timing-only ablations (break numerics)
DEXP = int(os.environ.get("K_DEXP", "0"))   # tiles of group g<SPLIT whose exp
# runs on the DVE via the f16 Schraudolph bit-trick instead of the ACT engine
DEXP_TILES = [(1, 0), (3, 0), (1, 1), (3, 1), (0, 0), (2, 0), (0, 1), (2, 1)]
SCH_A = 1024.0 / 0.6931471805599453          # log2(e) * 2^10
SCH_B = 15360.0 - 46.0                       # f16 exponent bias - Schraudolph C
OFF_WQ, OFF_WK, OFF_WV, OFF_WO = 0, 512, 1024, 1536
OFF_BO, OFF_MV, OFF_ID = 2048, 3072, 3104
OFF_BIAS = OFF_ID + 128
BLOB_COLS = OFF_BIAS + SPLIT * (H // 2) * 4 * Q
EB_COLS = (2 - SPLIT) * (H // 2) * 4 * Q
MD_COLS = SLOC * 4 * 32          # mask col f16, 32x-replicated per (s,b)
F32 = mybir.dt.float32
F32R = mybir.dt.float32r
F16 = mybir.dt.float16
I16 = mybir.dt.int16
EXP = mybir.ActivationFunctionType.Exp
MULT = mybir.AluOpType.mult
ADD = mybir.AluOpType.add

LAST_RESULT = None          # test.py reads exec_time/profile from here
_COMPILED = None


def build_nc(repeat=1):
    from contextlib import ExitStack

    nc = bacc.Bacc("TRN2", target_bir_lowering=False, debug=False,
                   enable_asserts=False, num_devices=NCORES)
    blob = nc.declare_dram_parameter("blob", [128, BLOB_COLS], F32R, isOutput=False)
    ebD = nc.declare_dram_parameter(
        "eb", [128, EB_COLS + MD_COLS], F16, isOutput=False)
    xqT = nc.declare_dram_parameter("xqT", [SLOC, C, Q], F32R, isOutput=False)
    xkvT = nc.declare_dram_parameter("xkvT", [SLOC, C, KV], F32R, isOutput=False)
    out = nc.declare_dram_parameter("out", [SLOC, Q, C], F32, isOutput=True)

    with tile.TileContext(nc) as tc, ExitStack() as ctx:
        singles = ctx.enter_context(tc.tile_pool(name="singles", bufs=1))
        qT_pool = ctx.enter_context(tc.tile_pool(name="qTp", bufs=2))
        kT_pool = ctx.enter_context(tc.tile_pool(name="kTp", bufs=2))
        v_pool = ctx.enter_context(tc.tile_pool(name="vp", bufs=2))
        p_pool = ctx.enter_context(tc.tile_pool(name="pp", bufs=6))
        recip_pool = ctx.enter_context(tc.tile_pool(name="rp", bufs=2))
        oT_pool = ctx.enter_context(tc.tile_pool(name="oTp", bufs=2))
        out_pool = ctx.enter_context(tc.tile_pool(name="outp", bufs=2))

        blob_t = singles.tile([128, BLOB_COLS], F32R)
        nc.sync.dma_start(out=blob_t[:], in_=blob[:])
        wq_t = blob_t[:, OFF_WQ:OFF_WQ + 512].rearrange("p (c m) -> p c m", c=2)
        wk_t = blob_t[:, OFF_WK:OFF_WK + 512].rearrange("p (c m) -> p c m", c=2)
        wv_t = blob_t[:, OFF_WV:OFF_WV + 512].rearrange("p (c m) -> p c m", c=2)
        wo_t = blob_t[:, OFF_WO:OFF_WO + 512].rearrange("p (c m) -> p c m", c=2)
        bo4_t = blob_t[:, OFF_BO:OFF_BO + 1024].bitcast(F32)
        maskv_t = blob_t[:, OFF_MV:OFF_MV + SLOC * 4].bitcast(F32)
        ident_t = blob_t[:, OFF_ID:OFF_ID + 128]
        if SPLIT:
            biasC = blob_t[:, OFF_BIAS:].rearrange(
                "p (h b q) -> p h b q", h=4 * SPLIT, b=4)
        if EB_COLS:
            eb_s = singles.tile([128, EB_COLS], F16)
            nc.sync.dma_start(out=eb_s[:], in_=ebD[:, :EB_COLS])
            eb_t = eb_s[:].rearrange(
                "p (h b q) -> p h b q", h=4 * (2 - SPLIT), b=4)
        md_s = singles.tile([128, SLOC, 4, 32], F16)
        nc.sync.dma_start(
            out=md_s[:], in_=ebD[:, EB_COLS:].rearrange(
                "p (s b m) -> p s b m", s=SLOC, b=4))
        zeros_t = singles.tile([128, 128], F16)
        nc.vector.memset(zeros_t[:], 0.0)
        xq_all = singles.tile([128, SLOC, 2, Q], F32R)
        nc.sync.dma_start(
            out=xq_all[:], in_=xqT[:].rearrange("s (c p) q -> p s c q", p=128))
        xkv_all = singles.tile([128, SLOC, 2, KV], F32R)
        nc.sync.dma_start(
            out=xkv_all[:], in_=xkvT[:].rearrange("s (c p) q -> p s c q", p=128))

        ps_sc = ctx.enter_context(
            tc.tile_pool(name="ps_sc", bufs=2, space="PSUM"))
        ps_ms = ctx.enter_context(
            tc.tile_pool(name="ps_ms", bufs=4, space="PSUM"))

        # ---------- projection pieces (each uses one 1-bank misc tile) ----
        def proj_qk_piece(s, hc, w_t, x_t, dstT):
            ps = ps_ms.tile([128, 512], F32, tag="ms", name=f"pj{hc}")
            for cc in range(2):
                nc.tensor.matmul(
                    ps[:], w_t[:, cc, 128 * hc:128 * hc + 128],
                    x_t[:, cc, :], start=(cc == 0), stop=(cc == 1))
            nc.vector.tensor_copy(dstT[:, hc, :], ps[:])

        def proj_v_piece(s, pr, v_t):
            xkv_t = xkv_all[:, s]
            ps = ps_ms.tile([128, 512], F32, tag="ms", name=f"pv{pr}")
            for bb in range(2):
                b2 = 2 * pr + bb
                for cc in range(2):
                    nc.tensor.matmul(
                        ps[:, 256 * bb:256 * bb + 256],
                        xkv_t[:, cc, 128 * b2:128 * b2 + 128],
                        wv_t[:, cc, :], start=(cc == 0), stop=(cc == 1))
            for bb in range(2):
                b2 = 2 * pr + bb
                nc.vector.tensor_scalar_mul(
                    v_t[:, b2, :], ps[:, 256 * bb:256 * bb + 256],
                    maskv_t[:, s * 4 + b2:s * 4 + b2 + 1])

        def emit_proj(s):
            qT_t = qT_pool.tile([128, 2, Q], F32R, tag="qT")
            kT_t = kT_pool.tile([128, 2, KV], F32R, tag="kT")
            v_t = v_pool.tile([128, 4, HD], F16, tag="v")
            return qT_t, kT_t, v_t

        # ---------- pipeline state ----------
        s_list = [s for _ in range(repeat) for s in range(SLOC)]
        nrow = len(s_list)

        # prologue: projections for row 0
        proj_tiles = {0: emit_proj(s_list[0])}
        for hc in range(2):
            proj_qk_piece(s_list[0], hc, wq_t, xq_all[:, s_list[0]], proj_tiles[0][0])
        for hc in range(2):
            proj_qk_piece(s_list[0], hc, wk_t, xkv_all[:, s_list[0]], proj_tiles[0][1])
        for pr in range(2):
            proj_v_piece(s_list[0], pr, proj_tiles[0][2])

        row_state = {}          # per-row: att/den/oT tiles

        def emit_scores(si, s, g, b, half, qT_t, kT_t):
            sc = ps_sc.tile([128, 1024], F32, tag="sc", name="sc")
            for jj in range(2):
                j = 2 * half + jj
                h = 4 * g + j
                if g < SPLIT and ABL != "bias":
                    nc.tensor.matmul(
                        sc[:, 512 * jj:512 * jj + 512],
                        ident_t[:], biasC[:, h, b, :],
                        start=True, stop=False, skip_group_check=True)
            for jj in range(2):
                j = 2 * half + jj
                nc.tensor.matmul(
                    sc[:, 512 * jj:512 * jj + 512],
                    kT_t[32 * j:32 * j + 32, g, 128 * b:128 * b + 128],
                    qT_t[32 * j:32 * j + 32, g, :],
                    start=(g >= SPLIT) or ABL == "bias", stop=True,
                    skip_group_check=True, tile_position=(32 * j, 0))
            p = p_pool.tile([128, 1024], F16, tag="p", name="p")
            if ABL == "exp":
                nc.vector.tensor_copy(
                    p[:, :8], sc[:, :8])  # tiny drain keeps sc ring moving
            elif g < SPLIT and (b, half) in DEXP_TILES[:DEXP]:
                # exp via f16 Schraudolph on the DVE: bitcast(round(A*x+B))
                nc.vector.tensor_scalar(
                    out=p[:], in0=sc[:], scalar1=SCH_A, scalar2=SCH_B,
                    op0=MULT, op1=ADD)
                pi = p_pool.tile([128, 1024], I16, tag="p", name="pi")
                nc.vector.tensor_scalar_max(pi[:], p[:], 0.0)
                p = pi.bitcast(F16)
            else:
                nc.scalar.activation(out=p[:], in_=sc[:], func=EXP)
            if g >= SPLIT and ABL != "mul":
                hh = 4 * (g - SPLIT) + 2 * half
                pm = p_pool.tile([128, 1024], F16, tag="p", name="pm")
                nc.vector.tensor_mul(
                    pm[:].rearrange("p (a q) -> p a q", a=2),
                    p[:].rearrange("p (a q) -> p a q", a=2),
                    eb_t[:, hh:hh + 2, b, :])
                p = pm
            return p

        def emit_attden(si, ps_, g, b, half, p, v_t):
            pb = b
            st = row_state[si]
            if ABL == "attden":
                if b == 0 and half == 0:
                    st[g] = (
                        ps_ms.tile([128, 512], F32, tag="ms", name="att"),
                        ps_ms.tile([128, 512], F32, tag="ms", name="den"))
                    att_t, den_t = st[g]
                    nc.tensor.matmul(att_t[:], zeros_t[:], p[:, :512],
                                     start=True, stop=(g == 9),
                                     skip_group_check=True)
                    nc.tensor.matmul(den_t[:], zeros_t[:], p[:, :512],
                                     start=True, stop=(g == 9),
                                     skip_group_check=True)
                return
            if b == 0 and half == 0:
                st[g] = (
                    ps_ms.tile([128, 512], F32, tag="ms", name="att"),
                    ps_ms.tile([128, 512], F32, tag="ms", name="den"))
                att_t, den_t = st[g]
                nc.tensor.matmul(att_t[:], zeros_t[:], p[:, :512],
                                 start=True, stop=False, skip_group_check=True)
                nc.tensor.matmul(den_t[:], zeros_t[:], p[:, :512],
                                 start=True, stop=False, skip_group_check=True)
            att_t, den_t = st[g]
            last = (b == 3 and half == 1)
            for jj in range(2):
                j = 2 * half + jj
                rhs = p[:, 512 * jj:512 * jj + 512]
                nc.tensor.matmul(
                    att_t[32 * j:32 * j + 32, :],
                    v_t[:, b, 32 * (4 * g + j):32 * (4 * g + j) + 32], rhs,
                    start=False, stop=(last and jj == 1),
                    skip_group_check=True, tile_position=(0, 32 * j))
            for jj in range(2):
                j = 2 * half + jj
                rhs = p[:, 512 * jj:512 * jj + 512]
                nc.tensor.matmul(
                    den_t[32 * j:32 * j + 32, :], md_s[:, ps_, pb, :], rhs,
                    start=False, stop=(last and jj == 1),
                    skip_group_check=True, tile_position=(0, 32 * j))

        def emit_norm(si, g):
            st = row_state[si]
            att_t, den_t = st[g]
            if g == 0:
                st["oT"] = oT_pool.tile([128, 1024], F32R, tag="oT", name="oT")
            oT_t = st["oT"]
            recip_t = recip_pool.tile([128, 512], F32, tag="recip")
            nc.vector.reciprocal_approx_fast(out=recip_t[:], in_=den_t[:])
            nc.vector.tensor_mul(oT_t[:, 512 * g:512 * g + 512],
                                 att_t[:], recip_t[:])

        def emit_outproj(si, s):
            st = row_state[si]
            oT_t = st["oT"]
            out_t = out_pool.tile([128, 4 * C], F32, tag="out")
            for pq in range(2):
                ps = ps_ms.tile([128, 512], F32, tag="ms", name=f"po{pq}")
                for qq in range(2):
                    qb = 2 * pq + qq
                    for c in range(2):
                        nc.tensor.matmul(
                            ps[:, 256 * qq:256 * qq + 256],
                            oT_t[:, 512 * c + 128 * qb:512 * c + 128 * qb + 128],
                            wo_t[:, c, :], start=(c == 0), stop=(c == 1))
                nc.vector.tensor_add(
                    out_t[:, 512 * pq:512 * pq + 512], ps[:],
                    bo4_t[:, 512 * pq:512 * pq + 512])
            nc.gpsimd.dma_start(
                out=out[s].rearrange("(b p) c -> p b c", p=128),
                in_=out_t[:].rearrange("p (b c) -> p b c", b=4))

        pend = None             # (si, s, g, b, half, p, v_t)

        def process_pend():
            nonlocal pend
            if pend is None:
                return
            psi, ps_, pg, pb, ph, pp, pv = pend
            emit_attden(psi, ps_, pg, pb, ph, pp, pv)
            pend = None
            u = ((pg * 4) + pb) * 2 + ph
            if u == 1 and psi > 0 and (psi - 1) in row_state:
                emit_outproj(psi - 1, s_list[psi - 1])
                del row_state[psi - 1]
            elif u == 7:
                emit_norm(psi, 0)
                if psi + 1 < nrow:
                    sn = s_list[psi + 1]
                    for hc in range(2):
                        proj_qk_piece(sn, hc, wq_t, xq_all[:, sn],
                                      proj_tiles[psi + 1][0])
            elif u == 11:
                if psi + 1 < nrow:
                    sn = s_list[psi + 1]
                    for hc in range(2):
                        proj_qk_piece(sn, hc, wk_t, xkv_all[:, sn],
                                      proj_tiles[psi + 1][1])
            elif u == 15:
                emit_norm(psi, 1)
                if psi + 1 < nrow:
                    sn = s_list[psi + 1]
                    for pr in range(2):
                        proj_v_piece(sn, pr, proj_tiles[psi + 1][2])

        for si, s in enumerate(s_list):
            row_state[si] = {}
            qT_t, kT_t, v_t = proj_tiles[si]
            if si + 1 < nrow:
                proj_tiles[si + 1] = emit_proj(s_list[si + 1])
            for g in range(2):
                for b in range(4):
                    for half in range(2):
                        p = emit_scores(si, s, g, b, half, qT_t, kT_t)
                        process_pend()
                        pend = (si, s, g, b, half, p, v_t)
            if si - 1 >= 0:
                del proj_tiles[si - 1]
        process_pend()
        emit_outproj(nrow - 1, s_list[nrow - 1])
        del row_state[nrow - 1]

    nc.compile()
    return nc


def _get_compiled():
    global _COMPILED
    if _COMPILED is None:
        _COMPILED = build_nc()
    return _COMPILED


def prep_in_maps(input_q, input_kv, mask, bias, Wq, Wkv, Wo, bo):
    input_q = np.asarray(input_q, dtype=np.float32)
    input_kv = np.asarray(input_kv, dtype=np.float32)
    mask = np.asarray(mask, dtype=np.float32)
    bias = np.asarray(bias, dtype=np.float32)
    Wq = np.asarray(Wq, dtype=np.float32)
    Wkv = np.asarray(Wkv, dtype=np.float32)
    Wo = np.asarray(Wo, dtype=np.float32)
    bo = np.asarray(bo, dtype=np.float32)

    # [h, kv, q] bias, packed as [p, h, b, q]; f32 for PE groups, exp-f16 rest
    biasT = np.transpose(bias[0, 0], (0, 2, 1))
    bias_pk = np.ascontiguousarray(
        biasT.reshape(H, 4, 128, Q).transpose(2, 0, 1, 3).reshape(128, H * 4 * Q))
    eb_pk = np.exp(bias_pk[:, 4 * SPLIT * 4 * Q:]).astype(np.float16)
    if EB_COLS == 0:
        eb_pk = np.zeros((128, 0), np.float16)

    def chunks2(w):  # [C, M] -> [p, (c m)] with 128-row C-chunks
        return w.reshape(2, 128, w.shape[1]).transpose(1, 0, 2).reshape(128, -1)

    wq_s = chunks2(Wq / np.sqrt(np.float32(D)))
    wk_pk = chunks2(Wkv[:, :HD])
    wv_pk = chunks2(Wkv[:, HD:])
    wo_pk = chunks2(Wo)
    bo4 = np.tile(bo[None, :], (128, 4))
    ident = np.eye(128, dtype=np.float32)

    in_maps = []
    for cid in range(NCORES):
        sl = slice(cid * SLOC, (cid + 1) * SLOC)
        xqT = np.ascontiguousarray(np.transpose(input_q[0, sl], (0, 2, 1)))
        xkvT = np.ascontiguousarray(np.transpose(input_kv[0, sl], (0, 2, 1)))
        m = mask[0, sl, 0, 0, :]                       # [SLOC, KV]
        maskcol = m.reshape(SLOC, 4, 128).transpose(2, 0, 1).reshape(128, SLOC * 4)
        md = np.ascontiguousarray(np.broadcast_to(
            maskcol.astype(np.float16)[:, :, None], (128, SLOC * 4, 32))
        ).reshape(128, MD_COLS)
        blob = np.zeros((128, BLOB_COLS), np.float32)
        blob[:, OFF_WQ:OFF_WQ + 512] = wq_s
        blob[:, OFF_WK:OFF_WK + 512] = wk_pk
        blob[:, OFF_WV:OFF_WV + 512] = wv_pk
        blob[:, OFF_WO:OFF_WO + 512] = wo_pk
        blob[:, OFF_BO:OFF_BO + 1024] = bo4
        blob[:, OFF_MV:OFF_MV + SLOC * 4] = maskcol
        blob[:, OFF_ID:OFF_ID + 128] = ident
        if SPLIT:
            blob[:, OFF_BIAS:] = bias_pk[:, :SPLIT * 4 * 4 * Q]
        in_maps.append(dict(
            blob=blob, eb=np.concatenate([eb_pk, md], axis=1),
            xqT=xqT, xkvT=xkvT))

    return in_maps


def kernel(input_q, input_kv, mask, bias, Wq, Wkv, Wo, bo):
    global LAST_RESULT
    nc = _get_compiled()
    in_maps = prep_in_maps(input_q, input_kv, mask, bias, Wq, Wkv, Wo, bo)
    trace = bool(int(os.environ.get("KERNEL_TRACE", "0")))
    LAST_RESULT = run_bass_kernel_spmd(
        nc, in_maps, list(range(NCORES)), trace=trace)
    outs = [LAST_RESULT.results[cid]["out"] for cid in range(NCORES)]
    full = np.concatenate(outs, axis=0)[None]          # [1, S, Q, C]
    return np.ascontiguousarray(full.astype(np.float32))


if __name__ == "__main__":
    rng = np.random.default_rng(0)
    demo = dict(
        input_q=rng.standard_normal((1, S, Q, C), dtype=np.float32),
        input_kv=rng.standard_normal((1, S, KV, C), dtype=np.float32),
        mask=np.ones((1, S, 1, 1, KV), np.float32),
        bias=rng.standard_normal((1, 1, H, Q, KV), dtype=np.float32) * 0.1,
        Wq=rng.standard_normal((C, HD), dtype=np.float32) * 0.06,
        Wkv=rng.standard_normal((C, 2 * HD), dtype=np.float32) * 0.05,
        Wo=rng.standard_normal((HD, C), dtype=np.float32) * 0.02,
        bo=np.zeros((C,), np.float32),
    )
    o = kernel(**demo)
    print("out", o.shape, o.dtype, float(np.abs(o).max()))
